# revision 1
# baseline (speedup 1.0000x reference)
"""Corr1d cost-volume kernel for Trainium2 (8 NeuronCores).

corr[b, d, h, x] = sum_c fL[b,c,h,x] * fR[b,c,h,x-d]  for x >= d, else 0.
Shapes: fL, fR = (4, 64, 256, 512) fp32; out = (4, 48, 256, 512) fp32.

Sharding: data-parallel over (batch, h-half): core i handles b = i//2,
h rows [128*(i%2), 128*(i%2)+128).

Per-core pipeline (per h row):
  - fp16 cast-load of fL/fR h-batches (SWDGE)
  - 4 banded matmuls (contract c=64 on partitions): lhsT = fL[c, x-block(128)],
    rhs = fR[c, window(176)] -> PSUM [128, 176] fp32
  - DVE copy PSUM -> SBUF fp16 data tile [128, 704]
  - gpsimd local_scatter with a constant per-partition index table: shears the
    diagonal band into a rect [128 x, 192 = 4 blocks x 48 d] (zero-filled)
  - 2 PE transposes [128, 96] -> PSUM [96, 128]
  - ACT copies -> fp32 assembly [96, NH*256]
  - 4 output DMAs per h-batch
"""
import numpy as np
from contextlib import ExitStack

import concourse.bass as bass
import concourse.tile as tile
import concourse.bacc as bacc
import concourse.mybir as mybir
from concourse import bass_utils
from concourse.ap import AP

B, C, H, W = 4, 64, 256, 512
D = 48
NCORES = 8
HH = H // 2            # h rows per core
NH = 16                # h rows per batch
NBATCH = HH // NH      # 16
WRHS = 192             # rhs window width (4 x 48 for the fold)
W0S = [0, 81, 209, 320]  # rhs window starts per x-block
NBLK = 4

fp16 = mybir.dt.float16
fp32 = mybir.dt.float32
i16 = mybir.dt.int16


def _make_tables():
    # band mask: mask[p, 192m + n] = 1 iff d = base_m + p - n in [0, 48)
    mask = np.zeros((128, NBLK * WRHS), dtype=np.float16)
    # rotation idx: folded col j of block m holds d = (base_m + p - j) mod 48
    idx1 = np.zeros((128, NBLK * D), dtype=np.int16)
    for m in range(NBLK):
        base = 128 * m - W0S[m]
        for p in range(128):
            for n in range(WRHS):
                if 0 <= base + p - n < D:
                    mask[p, WRHS * m + n] = 1.0
            for j in range(D):
                d = (base + p - j) % D
                idx1[p, D * m + j] = D * m + d
    parts = []
    for hi_ in range(NSC):
        t = idx1 + hi_ * NBLK * D
        parts.append(t)
    idx = np.concatenate(parts, axis=1)
    ident = np.eye(128, dtype=np.float16)
    return idx, mask, ident


NSC = 2                # h rows per local_scatter


def _build_nc():
    nc = bacc.Bacc("TRN2", target_bir_lowering=False, debug=False,
                   num_devices=NCORES)
    fL_d = nc.dram_tensor("fLc", [C, HH, W], fp16, kind="ExternalInput").ap()
    fR_d = nc.dram_tensor("fRc", [C, HH, W], fp16, kind="ExternalInput").ap()
    idx_d = nc.dram_tensor("idx", [128, NSC * NBLK * D], i16,
                           kind="ExternalInput").ap()
    mask_d = nc.dram_tensor("mask", [128, NBLK * WRHS], fp16,
                            kind="ExternalInput").ap()
    ident_d = nc.dram_tensor("ident", [128, 128], fp16, kind="ExternalInput").ap()
    out_d = nc.dram_tensor("outc", [D, HH, W], fp32, kind="ExternalOutput").ap()

    with tile.TileContext(nc) as tc, ExitStack() as ctx:
        const_pool = ctx.enter_context(tc.tile_pool(name="const", bufs=1))
        in_pool = ctx.enter_context(tc.tile_pool(name="inp", bufs=2))
        data_pool = ctx.enter_context(tc.tile_pool(name="data", bufs=6))
        band_pool = ctx.enter_context(tc.tile_pool(name="band", bufs=4))
        asm_pool = ctx.enter_context(tc.tile_pool(name="asm", bufs=2))
        mm_psum = ctx.enter_context(tc.tile_pool(name="mmps", bufs=5, space="PSUM"))
        tp_psum = ctx.enter_context(tc.tile_pool(name="tpps", bufs=3, space="PSUM"))

        idx_t = const_pool.tile([128, NSC * NBLK * D], i16)
        nc.sync.dma_start(idx_t[:], idx_d)
        mask_t = const_pool.tile([128, NBLK * WRHS], fp16)
        nc.sync.dma_start(mask_t[:], mask_d)
        ident_t = const_pool.tile([128, 128], fp16)
        nc.sync.dma_start(ident_t[:], ident_d)

        NHH = NH // 2  # h rows per partition-half
        # psum packing: 4 blocks of one h in 2 banks at these col offsets
        PS_OFF = [0, WRHS, 512, 512 + WRHS]
        for ib in range(NBATCH):
            h0 = ib * NH
            # h rows h0..h0+3 -> partitions 0:64, h0+4..h0+7 -> 64:128
            fl = in_pool.tile([128, NHH * W], fp16, tag="fl")
            fr = in_pool.tile([128, NHH * W], fp16, tag="fr")
            for half in range(2):
                nc.sync.dma_start(
                    fl[64 * half : 64 * half + 64, :]
                    .rearrange("c (h x) -> c h x", h=NHH),
                    fL_d[:, h0 + NHH * half : h0 + NHH * (half + 1), :],
                )
                nc.sync.dma_start(
                    fr[64 * half : 64 * half + 64, :]
                    .rearrange("c (h x) -> c h x", h=NHH),
                    fR_d[:, h0 + NHH * half : h0 + NHH * (half + 1), :],
                )

            asm = asm_pool.tile([96, NH * 256], fp32)

            def emit_transposes(band_, hp_):
                tp = tp_psum.tile([96, 512], fp16)
                for tt in range(4):
                    nc.tensor.transpose(
                        tp[:, 128 * tt : 128 * tt + 128],
                        band_[:, 96 * tt : 96 * tt + 96], ident_t[:]
                    )
                nc.scalar.copy(
                    asm[:].rearrange("q (hh x) -> q hh x", hh=NH)[:, hp_::NHH, :],
                    tp[:].rearrange("q (hb x) -> q hb x", hb=2),
                )

            pending = None
            for hp in range(NHH):
                # pair (hA, hB) = (hp, hp + NHH): hA on partitions 0:64,
                # hB on 64:128; PE row-group concurrency per block.
                # one PSUM bank per (hi, block-pair): [128, 512] holds 2 blocks
                pss = []
                for hi in range(2):
                    row = []
                    for bk in range(2):
                        ps = mm_psum.tile([128, 512], fp32, tag="mmps")
                        row.append(ps)
                    pss.append(row)
                for m in range(NBLK):
                    for hi in range(2):
                        pb = 64 * hi
                        nc.tensor.matmul(
                            pss[hi][m // 2][:, WRHS * (m % 2) :
                                            WRHS * (m % 2) + WRHS],
                            fl[pb : pb + 64,
                               hp * W + 128 * m : hp * W + 128 * m + 128],
                            fr[pb : pb + 64,
                               hp * W + W0S[m] : hp * W + W0S[m] + WRHS],
                            start=True,
                            stop=True,
                        )
                folded = data_pool.tile([128, 2 * NBLK * D], fp16, tag="folded")
                for hi in range(2):
                    # masked evacuation in k-major layout: psum col
                    # (m, 48k + j) -> data col 192k + 48m + j, so the fold
                    # adds below are flat contiguous halves.
                    data = data_pool.tile([128, NBLK * WRHS], fp16, tag="data")
                    for bk in range(2):
                        nc.vector.tensor_mul(
                            data[:]
                            .rearrange("p (k m j) -> p k m j", k=4, m=NBLK)
                            [:, :, 2 * bk : 2 * bk + 2, :]
                            .transpose([0, 2, 1, 3]),
                            pss[hi][bk][:, 0 : 2 * WRHS]
                            .rearrange("p (m k j) -> p m k j", m=2, k=4),
                            mask_t[:, 2 * WRHS * bk : 2 * WRHS * (bk + 1)]
                            .rearrange("p (m k j) -> p m k j", m=2, k=4),
                        )
                    # fold: sum the 4 k-planes (flat contiguous adds)
                    t1 = data_pool.tile([128, NBLK * 96], fp16, tag="t1")
                    with nc.allow_low_precision(reason="fold adds zeros"):
                        nc.vector.tensor_add(
                            t1[:], data[:, 0:384], data[:, 384:768]
                        )
                        nc.vector.tensor_add(
                            folded[:, NBLK * D * hi : NBLK * D * (hi + 1)],
                            t1[:, 0:192], t1[:, 192:384],
                        )
                band = band_pool.tile([128, 2 * NBLK * D], fp16)
                nc.gpsimd.local_scatter(
                    band[:], folded[:], idx_t[:],
                    channels=128, num_elems=2 * NBLK * D,
                    num_idxs=2 * NBLK * D,
                )
                tp = tp_psum.tile([96, 512], fp16)
                for tt in range(4):
                    nc.tensor.transpose(
                        tp[:, 128 * tt : 128 * tt + 128],
                        band[:, 96 * tt : 96 * tt + 96], ident_t[:]
                    )
                nc.scalar.copy(
                    asm[:].rearrange("q (hh x) -> q hh x", hh=NH)[:, hp::NHH, :],
                    tp[:].rearrange("q (hb x) -> q hb x", hb=2),
                )

            # output DMAs: asm[48*par + d, h*256 + t*128 + xin] ->
            #   out[d, h0+h, 256t + 128par + xin]
            for par in range(2):
                for t in range(2):
                    nc.sync.dma_start(
                        out_d[:, h0 : h0 + NH, 256 * t + 128 * par :
                              256 * t + 128 * par + 128],
                        asm[48 * par : 48 * par + 48, :]
                        .rearrange("d (h x) -> d h x", h=NH)[:, :, 128 * t : 128 * t + 128],
                    )

    nc.compile()
    return nc


_NC_CACHE = None


def _get_nc():
    global _NC_CACHE
    if _NC_CACHE is None:
        _NC_CACHE = _build_nc()
    return _NC_CACHE


def kernel(fL: np.ndarray, fR: np.ndarray) -> np.ndarray:
    fL = np.asarray(fL, dtype=np.float32)
    fR = np.asarray(fR, dtype=np.float32)
    nc = _get_nc()
    idx, mask, ident = _make_tables()

    in_maps = []
    for core in range(NCORES):
        b, half = divmod(core, 2)
        sl = np.s_[b, :, half * HH : half * HH + HH, :]
        in_maps.append({
            "fLc": fL[sl].astype(np.float16),
            "fRc": fR[sl].astype(np.float16),
            "idx": idx,
            "mask": mask,
            "ident": ident,
        })

    res = bass_utils.run_bass_kernel_spmd(nc, in_maps, core_ids=list(range(NCORES)))
    out = np.empty((B, D, H, W), dtype=np.float32)
    for core in range(NCORES):
        b, half = divmod(core, 2)
        out[b, :, half * HH : half * HH + HH, :] = res.results[core]["outc"]
    return out



# revision 4
# speedup vs baseline: 1.8441x; 1.8441x over previous
"""Corr1d cost-volume kernel for Trainium2 (8 NeuronCores), V2.

corr[b, d, h, x] = sum_c fL[b,c,h,x] * fR[b,c,h,x-d]  for x >= d, else 0.
Shapes: fL, fR = (4, 64, 256, 512) fp32; out = (4, 48, 256, 512) fp32.

Sharding: data-parallel over (batch, h-half): core i handles b = i//2,
h rows [128*(i%2), 128*(i%2)+128).

Per-core pipeline (per quad = 4 h rows):
  - 64-wide x-blocks; per h row, 8 banded matmuls [c=64 -> 64 x, 112 win]
    packed two-blocks-per-psum-tile on partition halves -> [128, 4*112]
    fp32 per row, 4 rows per 4-bank psum quad [128, 2048]
  - ACT: one raw copy psum -> SBUF fp16 [128, 4*448] (no mask)
  - DVE: band mask multiply (const 0/1 tile, 2x fp16) + 48-stride fold
    adds -> rotated-band tile F [128, 4*192] (dense = real output size)
  - 2 output DMAs per quad (768B lines) into a dump tensor
Host: un-rotates the band with a precomputed numpy gather (free) and
assembles the fp32 output. x<d entries are zero via the mask (no valid
source column), matching the reference.
"""
import numpy as np
from contextlib import ExitStack

import concourse.bass as bass
import concourse.tile as tile
import concourse.bacc as bacc
import concourse.mybir as mybir
from concourse import bass_utils
from concourse.ap import AP

B, C, H, W = 4, 64, 256, 512
D = 48
NCORES = 8
HH = H // 2            # h rows per core
NH = 16                # h rows per load batch
NBATCH = HH // NH      # 8
WIN = 112              # rhs window width per 64-block
GB = 64                # x-block width
NBLK = W // GB         # 8 blocks per h row
# window start per block: 64b-47 clipped into [0, W-WIN]
SB = [max(0, min(64 * b - 47, W - WIN)) for b in range(NBLK)]

fp16 = mybir.dt.float16
fp32 = mybir.dt.float32


def _make_mask():
    # mask[p, 112t + j] = 1 iff d = x - SB[b] - j in [0, 48),
    # with p = 64H + u, b = 2t + H, x = 64b + u. Replicated x4 (quad rows).
    base = np.zeros((128, 4 * WIN), dtype=np.float16)
    for p in range(128):
        Hc, u = divmod(p, GB)
        for t in range(4):
            b = 2 * t + Hc
            x = GB * b + u
            for j in range(WIN):
                if 0 <= x - SB[b] - j < D:
                    base[p, WIN * t + j] = 1.0
    return np.tile(base, (1, 4))


def _build_nc():
    nc = bacc.Bacc("TRN2", target_bir_lowering=False, debug=False,
                   num_devices=NCORES)
    fL_d = nc.dram_tensor("fLc", [C, HH, W], fp16, kind="ExternalInput").ap()
    fR_d = nc.dram_tensor("fRc", [C, HH, W], fp16, kind="ExternalInput").ap()
    mask_d = nc.dram_tensor("maskc", [128, 4 * 4 * WIN], fp16,
                            kind="ExternalInput").ap()
    dump_d = nc.dram_tensor("dump", [NBATCH * 8, 128, 2 * 4 * D], fp16,
                            kind="ExternalOutput").ap()

    with tile.TileContext(nc) as tc, ExitStack() as ctx:
        const_pool = ctx.enter_context(tc.tile_pool(name="const", bufs=1))
        in_pool = ctx.enter_context(tc.tile_pool(name="inp", bufs=2))
        raw_pool = ctx.enter_context(tc.tile_pool(name="raw", bufs=3))
        msk_pool = ctx.enter_context(tc.tile_pool(name="msk", bufs=3))
        f_pool = ctx.enter_context(tc.tile_pool(name="fold", bufs=3))
        mm_psum = ctx.enter_context(tc.tile_pool(name="mmps", bufs=2, space="PSUM"))

        mask_t = const_pool.tile([128, 4 * 4 * WIN], fp16)
        nc.sync.dma_start(mask_t[:], mask_d)

        NHH = NH // 2
        for ib in range(NBATCH):
            h0 = ib * NH
            # h rows h0..h0+7 -> partitions 0:64, h0+8..h0+15 -> 64:128
            fl = in_pool.tile([128, NHH * W], fp16, tag="fl")
            fr = in_pool.tile([128, NHH * W], fp16, tag="fr")
            for half in range(2):
                nc.sync.dma_start(
                    fl[64 * half : 64 * half + 64, :]
                    .rearrange("c (h x) -> c h x", h=NHH),
                    fL_d[:, h0 + NHH * half : h0 + NHH * (half + 1), :],
                )
                nc.sync.dma_start(
                    fr[64 * half : 64 * half + 64, :]
                    .rearrange("c (h x) -> c h x", h=NHH),
                    fR_d[:, h0 + NHH * half : h0 + NHH * (half + 1), :],
                )

            for hpb in range(4):
                # quad = pairs (2*hpb, 2*hpb+1) x hi in {0,1}; psum row
                # q = 2*pr + hi at cols [512q, 512q+448)
                ps = mm_psum.tile([128, 2048], fp32)
                for pr in range(2):
                    hp = 2 * hpb + pr
                    for t in range(4):
                        for Hc in range(2):
                            b = 2 * t + Hc
                            for hi in range(2):
                                q = 2 * pr + hi
                                nc.tensor.matmul(
                                    ps[64 * Hc : 64 * Hc + 64,
                                       512 * q + WIN * t : 512 * q + WIN * (t + 1)],
                                    fl[64 * hi : 64 * hi + 64,
                                       W * hp + GB * b : W * hp + GB * b + GB],
                                    fr[64 * hi : 64 * hi + 64,
                                       W * hp + SB[b] : W * hp + SB[b] + WIN],
                                    start=True,
                                    stop=True,
                                )
                # raw evacuation (ACT): psum fp32 -> SBUF fp16, no mask
                raw = raw_pool.tile([128, 4 * 4 * WIN], fp16)
                nc.scalar.copy(
                    raw[:].rearrange("p (q c) -> p q c", q=4),
                    ps[:].rearrange("p (q c) -> p q c", q=4)[:, :, 0 : 4 * WIN],
                )
                # band mask (DVE, 2x fp16)
                msk = msk_pool.tile([128, 4 * 4 * WIN], fp16)
                nc.vector.tensor_mul(msk[:], raw[:], mask_t[:])
                # fold 112 -> 48 (exactly one nonzero plane per output col)
                F = f_pool.tile([128, 4 * 4 * D], fp16)
                Tv = msk[:].rearrange("p (q t j) -> p q t j", q=4, t=4)
                Fv = F[:].rearrange("p (q t j) -> p q t j", q=4, t=4)
                with nc.allow_low_precision(reason="fold adds zeros"):
                    nc.vector.tensor_add(
                        Fv, Tv[:, :, :, 0:D], Tv[:, :, :, D : 2 * D]
                    )
                    nc.vector.tensor_add(
                        Fv[:, :, :, 0 : WIN - 2 * D],
                        Fv[:, :, :, 0 : WIN - 2 * D],
                        Tv[:, :, :, 2 * D : WIN],
                    )
                for pr in range(2):
                    hp = 2 * hpb + pr
                    nc.sync.dma_start(
                        dump_d[8 * ib + hp],
                        F[:, 2 * 4 * D * pr : 2 * 4 * D * (pr + 1)],
                    )

    nc.compile()
    return nc


_NC_CACHE = None


def _get_nc():
    global _NC_CACHE
    if _NC_CACHE is None:
        _NC_CACHE = _build_nc()
    return _NC_CACHE


def make_in_maps(fL, fR):
    maskc = _make_mask()
    in_maps = []
    for core in range(NCORES):
        b, half = divmod(core, 2)
        sl = np.s_[b, :, half * HH : half * HH + HH, :]
        in_maps.append({
            "fLc": fL[sl].astype(np.float16),
            "fRc": fR[sl].astype(np.float16),
            "maskc": maskc,
        })
    return in_maps


_GATHER_CACHE = None


def _gather_tables():
    # out[d, h, x] = dump[ib, hp, P[x], 192*hi + CB[d, x]] * VALID[d, x]
    global _GATHER_CACHE
    if _GATHER_CACHE is None:
        xs = np.arange(W)
        ds = np.arange(D)
        bx = xs // GB
        ux = xs % GB
        Px = 64 * (bx % 2) + ux                          # [W]
        jabs = xs[None, :] - np.asarray(SB)[bx][None, :] - ds[:, None]  # [D, W]
        valid = (jabs >= 0) & (jabs < WIN)
        tb = bx // 2
        cb = D * tb[None, :] + np.where(valid, jabs, 0) % D             # [D, W]
        FI = Px[None, :] * (4 * D) + cb                  # [D, W] into [p, 192]
        _GATHER_CACHE = (FI.astype(np.int64), valid.astype(np.float32))
    return _GATHER_CACHE


def kernel(fL: np.ndarray, fR: np.ndarray) -> np.ndarray:
    fL = np.asarray(fL, dtype=np.float32)
    fR = np.asarray(fR, dtype=np.float32)
    nc = _get_nc()
    in_maps = make_in_maps(fL, fR)
    res = bass_utils.run_bass_kernel_spmd(nc, in_maps, core_ids=list(range(NCORES)))

    FI, valid = _gather_tables()
    out = np.empty((B, D, H, W), dtype=np.float32)
    for core in range(NCORES):
        b, half = divmod(core, 2)
        dump = res.results[core]["dump"]                 # [64, 128, 384]
        # [ib, hp, p, hi, c] -> [h = (ib, hi, hp), p*192 + c]
        arr = (dump.reshape(NBATCH, 8, 128, 2, 4 * D)
               .transpose(0, 3, 1, 2, 4)
               .reshape(HH, 128 * 4 * D))
        g = arr[:, FI.reshape(-1)].reshape(HH, D, W).astype(np.float32)
        g *= valid[None, :, :]
        out[b, :, half * HH : half * HH + HH, :] = g.transpose(1, 0, 2)
    return out


# revision 8
# speedup vs baseline: 2.0340x; 1.1030x over previous
"""Corr1d cost-volume kernel for Trainium2 (8 NeuronCores), V2.

corr[b, d, h, x] = sum_c fL[b,c,h,x] * fR[b,c,h,x-d]  for x >= d, else 0.
Shapes: fL, fR = (4, 64, 256, 512) fp32; out = (4, 48, 256, 512) fp32.

Sharding: data-parallel over (batch, h-half): core i handles b = i//2,
h rows [128*(i%2), 128*(i%2)+128).

Per-core pipeline (per quad = 4 h rows):
  - 64-wide x-blocks; per h row, 8 banded matmuls [c=64 -> 64 x, 112 win]
    packed two-blocks-per-psum-tile on partition halves -> [128, 4*112]
    fp32 per row, 4 rows per 4-bank psum quad [128, 2048]
  - ACT: one raw copy psum -> SBUF fp16 [128, 4*448] (no mask)
  - DVE: band mask multiply (const 0/1 tile, 2x fp16) + 48-stride fold
    adds -> rotated-band tile F [128, 4*192] (dense = real output size)
  - 2 output DMAs per quad (768B lines) into a dump tensor
Host: un-rotates the band with a precomputed numpy gather (free) and
assembles the fp32 output. x<d entries are zero via the mask (no valid
source column), matching the reference.
"""
import numpy as np
from contextlib import ExitStack

import concourse.bass as bass
import concourse.tile as tile
import concourse.bacc as bacc
import concourse.mybir as mybir
from concourse import bass_utils
from concourse.ap import AP

B, C, H, W = 4, 64, 256, 512
D = 48
NCORES = 8
HH = H // 2            # h rows per core
NH = 16                # h rows per load batch
NBATCH = HH // NH      # 8
WIN = 112              # rhs window width per 64-block
GB = 64                # x-block width
NBLK = W // GB         # 8 blocks per h row
# window start per block: 64b-47 clipped into [0, W-WIN]
SB = [max(0, min(64 * b - 47, W - WIN)) for b in range(NBLK)]

fp16 = mybir.dt.float16
fp32 = mybir.dt.float32


def _make_mask():
    # mask[p, 112t + j] = 1 iff d = x - SB[b] - j in [0, 48),
    # with p = 64H + u, b = 2t + H, x = 64b + u. Replicated x4 (quad rows).
    base = np.zeros((128, 4 * WIN), dtype=np.float16)
    for p in range(128):
        Hc, u = divmod(p, GB)
        for t in range(4):
            b = 2 * t + Hc
            x = GB * b + u
            for j in range(WIN):
                if 0 <= x - SB[b] - j < D:
                    base[p, WIN * t + j] = 1.0
    return np.tile(base, (1, 4))


def _build_nc():
    nc = bacc.Bacc("TRN2", target_bir_lowering=False, debug=False,
                   num_devices=NCORES)
    fL_d = nc.dram_tensor("fLc", [C, HH, W], fp16, kind="ExternalInput").ap()
    fR_d = nc.dram_tensor("fRc", [C, HH, W], fp16, kind="ExternalInput").ap()
    mask_d = nc.dram_tensor("maskc", [128, 4 * 4 * WIN], fp16,
                            kind="ExternalInput").ap()
    # per load-batch: [p, (hpb, pr, hi, t, j')] -> 6KB DMA lines
    dump_d = nc.dram_tensor("dump", [NBATCH, 128, 16 * 4 * D], fp16,
                            kind="ExternalOutput").ap()

    with tile.TileContext(nc) as tc, ExitStack() as ctx:
        const_pool = ctx.enter_context(tc.tile_pool(name="const", bufs=1))
        in_pool = ctx.enter_context(tc.tile_pool(name="inp", bufs=2))
        raw_pool = ctx.enter_context(tc.tile_pool(name="raw", bufs=3))
        msk_pool = ctx.enter_context(tc.tile_pool(name="msk", bufs=3))
        f_pool = ctx.enter_context(tc.tile_pool(name="fold", bufs=3))
        mm_psum = ctx.enter_context(tc.tile_pool(name="mmps", bufs=2, space="PSUM"))

        mask_t = const_pool.tile([128, 4 * 4 * WIN], fp16)
        nc.sync.dma_start(mask_t[:], mask_d)

        NHH = NH // 2
        for ib in range(NBATCH):
            h0 = ib * NH
            # h rows h0..h0+7 -> partitions 0:64, h0+8..h0+15 -> 64:128
            fl = in_pool.tile([128, NHH * W], fp16, tag="fl")
            fr = in_pool.tile([128, NHH * W], fp16, tag="fr")
            for half in range(2):
                nc.sync.dma_start(
                    fl[64 * half : 64 * half + 64, :]
                    .rearrange("c (h x) -> c h x", h=NHH),
                    fL_d[:, h0 + NHH * half : h0 + NHH * (half + 1), :],
                )
                nc.sync.dma_start(
                    fr[64 * half : 64 * half + 64, :]
                    .rearrange("c (h x) -> c h x", h=NHH),
                    fR_d[:, h0 + NHH * half : h0 + NHH * (half + 1), :],
                )

            F = f_pool.tile([128, 16 * 4 * D], fp16)
            for hpb in range(4):
                # quad = pairs (2*hpb, 2*hpb+1) x hi in {0,1}; psum row
                # q = 2*pr + hi at cols [512q, 512q+448)
                ps = mm_psum.tile([128, 2048], fp32)
                for pr in range(2):
                    hp = 2 * hpb + pr
                    for t in range(4):
                        for Hc in range(2):
                            b = 2 * t + Hc
                            for hi in range(2):
                                q = 2 * pr + hi
                                nc.tensor.matmul(
                                    ps[64 * Hc : 64 * Hc + 64,
                                       512 * q + WIN * t : 512 * q + WIN * (t + 1)],
                                    fl[64 * hi : 64 * hi + 64,
                                       W * hp + GB * b : W * hp + GB * b + GB],
                                    fr[64 * hi : 64 * hi + 64,
                                       W * hp + SB[b] : W * hp + SB[b] + WIN],
                                    start=True,
                                    stop=True,
                                )
                # raw evacuation (ACT): psum fp32 -> SBUF fp16, no mask
                raw = raw_pool.tile([128, 4 * 4 * WIN], fp16)
                nc.scalar.copy(
                    raw[:].rearrange("p (q c) -> p q c", q=4),
                    ps[:].rearrange("p (q c) -> p q c", q=4)[:, :, 0 : 4 * WIN],
                )
                # band mask (DVE, 2x fp16)
                msk = msk_pool.tile([128, 4 * 4 * WIN], fp16)
                nc.vector.tensor_mul(msk[:], raw[:], mask_t[:])
                # fold 112 -> 48 (exactly one nonzero plane per output col)
                Tv = msk[:].rearrange("p (q t j) -> p q t j", q=4, t=4)
                Fv = (F[:, 4 * 4 * D * hpb : 4 * 4 * D * (hpb + 1)]
                      .rearrange("p (q t j) -> p q t j", q=4, t=4))
                with nc.allow_low_precision(reason="fold adds zeros"):
                    nc.vector.tensor_add(
                        Fv, Tv[:, :, :, 0:D], Tv[:, :, :, D : 2 * D]
                    )
                    nc.vector.tensor_add(
                        Fv[:, :, :, 0 : WIN - 2 * D],
                        Fv[:, :, :, 0 : WIN - 2 * D],
                        Tv[:, :, :, 2 * D : WIN],
                    )
            nc.sync.dma_start(dump_d[ib], F[:])

    nc.compile()
    return nc


_NC_CACHE = None


def _get_nc():
    global _NC_CACHE
    if _NC_CACHE is None:
        _NC_CACHE = _build_nc()
    return _NC_CACHE


def make_in_maps(fL, fR):
    maskc = _make_mask()
    in_maps = []
    for core in range(NCORES):
        b, half = divmod(core, 2)
        sl = np.s_[b, :, half * HH : half * HH + HH, :]
        in_maps.append({
            "fLc": fL[sl].astype(np.float16),
            "fRc": fR[sl].astype(np.float16),
            "maskc": maskc,
        })
    return in_maps


_GATHER_CACHE = None


def _gather_tables():
    # out[d, h, x] = dump[ib, hp, P[x], 192*hi + CB[d, x]] * VALID[d, x]
    global _GATHER_CACHE
    if _GATHER_CACHE is None:
        xs = np.arange(W)
        ds = np.arange(D)
        bx = xs // GB
        ux = xs % GB
        Px = 64 * (bx % 2) + ux                          # [W]
        jabs = xs[None, :] - np.asarray(SB)[bx][None, :] - ds[:, None]  # [D, W]
        valid = (jabs >= 0) & (jabs < WIN)
        tb = bx // 2
        cb = D * tb[None, :] + np.where(valid, jabs, 0) % D             # [D, W]
        FI = Px[None, :] * (4 * D) + cb                  # [D, W] into [p, 192]
        _GATHER_CACHE = (FI.astype(np.int64), valid.astype(np.float32))
    return _GATHER_CACHE


def kernel(fL: np.ndarray, fR: np.ndarray) -> np.ndarray:
    fL = np.asarray(fL, dtype=np.float32)
    fR = np.asarray(fR, dtype=np.float32)
    nc = _get_nc()
    in_maps = make_in_maps(fL, fR)
    res = bass_utils.run_bass_kernel_spmd(nc, in_maps, core_ids=list(range(NCORES)))

    FI, valid = _gather_tables()
    out = np.empty((B, D, H, W), dtype=np.float32)
    for core in range(NCORES):
        b, half = divmod(core, 2)
        dump = res.results[core]["dump"]                 # [8, 128, 3072]
        # [ib, p, hpb, pr, hi, c] -> [h = (ib, hi, hpb, pr), p*192 + c]
        arr = (dump.reshape(NBATCH, 128, 4, 2, 2, 4 * D)
               .transpose(0, 4, 2, 3, 1, 5)
               .reshape(HH, 128 * 4 * D))
        g = arr[:, FI.reshape(-1)].reshape(HH, D, W).astype(np.float32)
        g *= valid[None, :, :]
        out[b, :, half * HH : half * HH + HH, :] = g.transpose(1, 0, 2)
    return out


# revision 11
# speedup vs baseline: 2.1806x; 1.0721x over previous
"""Corr1d cost-volume kernel for Trainium2 (8 NeuronCores), V2.

corr[b, d, h, x] = sum_c fL[b,c,h,x] * fR[b,c,h,x-d]  for x >= d, else 0.
Shapes: fL, fR = (4, 64, 256, 512) fp32; out = (4, 48, 256, 512) fp32.

Sharding: data-parallel over (batch, h-half): core i handles b = i//2,
h rows [128*(i%2), 128*(i%2)+128).

Per-core pipeline (per quad = 4 h rows):
  - 64-wide x-blocks; per h row, 8 banded matmuls [c=64 -> 64 x, 112 win]
    packed two-blocks-per-psum-tile on partition halves -> [128, 4*112]
    fp32 per row, 4 rows per 4-bank psum quad [128, 2048]
  - ACT: one raw copy psum -> SBUF fp16 [128, 4*448] (no mask)
  - DVE: band mask multiply (const 0/1 tile, 2x fp16) + 48-stride fold
    adds -> rotated-band tile F [128, 4*192] (dense = real output size)
  - 2 output DMAs per quad (768B lines) into a dump tensor
Host: un-rotates the band with a precomputed numpy gather (free) and
assembles the fp32 output. x<d entries are zero via the mask (no valid
source column), matching the reference.
"""
import numpy as np
from contextlib import ExitStack

import concourse.bass as bass
import concourse.tile as tile
import concourse.bacc as bacc
import concourse.mybir as mybir
from concourse import bass_utils
from concourse.ap import AP

B, C, H, W = 4, 64, 256, 512
D = 48
NCORES = 8
HH = H // 2            # h rows per core
NH = 16                # h rows per load batch
NBATCH = HH // NH      # 8
WIN = 112              # rhs window width per 64-block
GB = 64                # x-block width
NBLK = W // GB         # 8 blocks per h row
# window start per block: 64b-47 clipped into [0, W-WIN]
SB = [max(0, min(64 * b - 47, W - WIN)) for b in range(NBLK)]

fp16 = mybir.dt.float16
fp32 = mybir.dt.float32


def _make_mask():
    # mask[p, 112t + j] = 1 iff d = x - SB[b] - j in [0, 48),
    # with p = 64H + u, b = 2t + H, x = 64b + u. Replicated x4 (quad rows).
    base = np.zeros((128, 4 * WIN), dtype=np.float16)
    for p in range(128):
        Hc, u = divmod(p, GB)
        for t in range(4):
            b = 2 * t + Hc
            x = GB * b + u
            for j in range(WIN):
                if 0 <= x - SB[b] - j < D:
                    base[p, WIN * t + j] = 1.0
    return np.tile(base, (1, 4))


def _build_nc():
    nc = bacc.Bacc("TRN2", target_bir_lowering=False, debug=False,
                   num_devices=NCORES)
    fL_d = nc.dram_tensor("fLc", [C, HH, W], fp16, kind="ExternalInput").ap()
    fR_d = nc.dram_tensor("fRc", [C, HH, W], fp16, kind="ExternalInput").ap()
    mask_d = nc.dram_tensor("maskc", [128, 4 * 4 * WIN], fp16,
                            kind="ExternalInput").ap()
    # per load-batch: [p, (hpb, pr, hi, t, j')] -> 6KB DMA lines
    dump_d = nc.dram_tensor("dump", [NBATCH, 128, 16 * 4 * D], fp16,
                            kind="ExternalOutput").ap()

    with tile.TileContext(nc) as tc, ExitStack() as ctx:
        const_pool = ctx.enter_context(tc.tile_pool(name="const", bufs=1))
        in_pool = ctx.enter_context(tc.tile_pool(name="inp", bufs=3))
        raw_pool = ctx.enter_context(tc.tile_pool(name="raw", bufs=3))
        msk_pool = ctx.enter_context(tc.tile_pool(name="msk", bufs=3))
        f_pool = ctx.enter_context(tc.tile_pool(name="fold", bufs=3))
        mm_psum = ctx.enter_context(tc.tile_pool(name="mmps", bufs=2, space="PSUM"))

        mask_t = const_pool.tile([128, 4 * 4 * WIN], fp16)
        nc.gpsimd.dma_start(mask_t[:], mask_d)

        NHH = NH // 2
        for ib in range(NBATCH):
            h0 = ib * NH
            # h rows h0..h0+7 -> partitions 0:64, h0+8..h0+15 -> 64:128
            fl = in_pool.tile([128, NHH * W], fp16, tag="fl")
            fr = in_pool.tile([128, NHH * W], fp16, tag="fr")
            for half in range(2):
                nc.sync.dma_start(
                    fl[64 * half : 64 * half + 64, :]
                    .rearrange("c (h x) -> c h x", h=NHH),
                    fL_d[:, h0 + NHH * half : h0 + NHH * (half + 1), :],
                )
                nc.gpsimd.dma_start(
                    fr[64 * half : 64 * half + 64, :]
                    .rearrange("c (h x) -> c h x", h=NHH),
                    fR_d[:, h0 + NHH * half : h0 + NHH * (half + 1), :],
                )

            F = f_pool.tile([128, 16 * 4 * D], fp16)
            for hpb in range(4):
                # quad = pairs (2*hpb, 2*hpb+1) x hi in {0,1}; psum row
                # q = 2*pr + hi at cols [512q, 512q+448)
                ps = mm_psum.tile([128, 2048], fp32)
                for pr in range(2):
                    hp = 2 * hpb + pr
                    for t in range(4):
                        for Hc in range(2):
                            b = 2 * t + Hc
                            for hi in range(2):
                                q = 2 * pr + hi
                                nc.tensor.matmul(
                                    ps[64 * Hc : 64 * Hc + 64,
                                       512 * q + WIN * t : 512 * q + WIN * (t + 1)],
                                    fl[64 * hi : 64 * hi + 64,
                                       W * hp + GB * b : W * hp + GB * b + GB],
                                    fr[64 * hi : 64 * hi + 64,
                                       W * hp + SB[b] : W * hp + SB[b] + WIN],
                                    start=True,
                                    stop=True,
                                )
                # raw evacuation (ACT): psum fp32 -> SBUF fp16, no mask
                raw = raw_pool.tile([128, 4 * 4 * WIN], fp16)
                nc.scalar.copy(
                    raw[:].rearrange("p (q c) -> p q c", q=4),
                    ps[:].rearrange("p (q c) -> p q c", q=4)[:, :, 0 : 4 * WIN],
                )
                # band mask (DVE, 2x fp16)
                msk = msk_pool.tile([128, 4 * 4 * WIN], fp16)
                nc.vector.tensor_mul(msk[:], raw[:], mask_t[:])
                # fold 112 -> 48 (exactly one nonzero plane per output col)
                Tv = msk[:].rearrange("p (q t j) -> p q t j", q=4, t=4)
                Fv = (F[:, 4 * 4 * D * hpb : 4 * 4 * D * (hpb + 1)]
                      .rearrange("p (q t j) -> p q t j", q=4, t=4))
                with nc.allow_low_precision(reason="fold adds zeros"):
                    nc.vector.tensor_add(
                        Fv, Tv[:, :, :, 0:D], Tv[:, :, :, D : 2 * D]
                    )
                    nc.vector.tensor_add(
                        Fv[:, :, :, 0 : WIN - 2 * D],
                        Fv[:, :, :, 0 : WIN - 2 * D],
                        Tv[:, :, :, 2 * D : WIN],
                    )
            nc.sync.dma_start(dump_d[ib], F[:])

    nc.compile()
    return nc


_NC_CACHE = None


def _get_nc():
    global _NC_CACHE
    if _NC_CACHE is None:
        _NC_CACHE = _build_nc()
    return _NC_CACHE


def make_in_maps(fL, fR):
    maskc = _make_mask()
    in_maps = []
    for core in range(NCORES):
        b, half = divmod(core, 2)
        sl = np.s_[b, :, half * HH : half * HH + HH, :]
        in_maps.append({
            "fLc": fL[sl].astype(np.float16),
            "fRc": fR[sl].astype(np.float16),
            "maskc": maskc,
        })
    return in_maps


_GATHER_CACHE = None


def _gather_tables():
    # out[d, h, x] = dump[ib, hp, P[x], 192*hi + CB[d, x]] * VALID[d, x]
    global _GATHER_CACHE
    if _GATHER_CACHE is None:
        xs = np.arange(W)
        ds = np.arange(D)
        bx = xs // GB
        ux = xs % GB
        Px = 64 * (bx % 2) + ux                          # [W]
        jabs = xs[None, :] - np.asarray(SB)[bx][None, :] - ds[:, None]  # [D, W]
        valid = (jabs >= 0) & (jabs < WIN)
        tb = bx // 2
        cb = D * tb[None, :] + np.where(valid, jabs, 0) % D             # [D, W]
        FI = Px[None, :] * (4 * D) + cb                  # [D, W] into [p, 192]
        _GATHER_CACHE = (FI.astype(np.int64), valid.astype(np.float32))
    return _GATHER_CACHE


def kernel(fL: np.ndarray, fR: np.ndarray) -> np.ndarray:
    fL = np.asarray(fL, dtype=np.float32)
    fR = np.asarray(fR, dtype=np.float32)
    nc = _get_nc()
    in_maps = make_in_maps(fL, fR)
    res = bass_utils.run_bass_kernel_spmd(nc, in_maps, core_ids=list(range(NCORES)))

    FI, valid = _gather_tables()
    out = np.empty((B, D, H, W), dtype=np.float32)
    for core in range(NCORES):
        b, half = divmod(core, 2)
        dump = res.results[core]["dump"]                 # [8, 128, 3072]
        # [ib, p, hpb, pr, hi, c] -> [h = (ib, hi, hpb, pr), p*192 + c]
        arr = (dump.reshape(NBATCH, 128, 4, 2, 2, 4 * D)
               .transpose(0, 4, 2, 3, 1, 5)
               .reshape(HH, 128 * 4 * D))
        g = arr[:, FI.reshape(-1)].reshape(HH, D, W).astype(np.float32)
        g *= valid[None, :, :]
        out[b, :, half * HH : half * HH + HH, :] = g.transpose(1, 0, 2)
    return out


# revision 15
# speedup vs baseline: 2.2979x; 1.0538x over previous
"""Corr1d cost-volume kernel for Trainium2 (8 NeuronCores), V2.

corr[b, d, h, x] = sum_c fL[b,c,h,x] * fR[b,c,h,x-d]  for x >= d, else 0.
Shapes: fL, fR = (4, 64, 256, 512) fp32; out = (4, 48, 256, 512) fp32.

Sharding: data-parallel over (batch, h-half): core i handles b = i//2,
h rows [128*(i%2), 128*(i%2)+128).

Per-core pipeline (per quad = 4 h rows):
  - 64-wide x-blocks; per h row, 8 banded matmuls [c=64 -> 64 x, 112 win]
    packed two-blocks-per-psum-tile on partition halves -> [128, 4*112]
    fp32 per row, 4 rows per 4-bank psum quad [128, 2048]
  - ACT: one raw copy psum -> SBUF fp16 [128, 4*448] (no mask)
  - DVE: band mask multiply (const 0/1 tile, 2x fp16) + 48-stride fold
    adds -> rotated-band tile F [128, 4*192] (dense = real output size)
  - 2 output DMAs per quad (768B lines) into a dump tensor
Host: un-rotates the band with a precomputed numpy gather (free) and
assembles the fp32 output. x<d entries are zero via the mask (no valid
source column), matching the reference.
"""
import numpy as np
from contextlib import ExitStack

import concourse.bass as bass
import concourse.tile as tile
import concourse.bacc as bacc
import concourse.mybir as mybir
from concourse import bass_utils
from concourse.ap import AP

B, C, H, W = 4, 64, 256, 512
D = 48
NCORES = 8
HH = H // 2            # h rows per core
NH = 16                # h rows per load batch
NBATCH = HH // NH      # 8
WIN = 112              # rhs window width per 64-block
GB = 64                # x-block width
NBLK = W // GB         # 8 blocks per h row
# window start per block: 64b-47 clipped into [0, W-WIN]
SB = [max(0, min(64 * b - 47, W - WIN)) for b in range(NBLK)]

fp16 = mybir.dt.float16
fp32 = mybir.dt.float32


def _make_mask():
    # mask[p, 112t + j] = 1 iff d = x - SB[b] - j in [0, 48),
    # with p = 64H + u, b = 2t + H, x = 64b + u. Replicated x4 (quad rows).
    base = np.zeros((128, 4 * WIN), dtype=np.float16)
    for p in range(128):
        Hc, u = divmod(p, GB)
        for t in range(4):
            b = 2 * t + Hc
            x = GB * b + u
            for j in range(WIN):
                if 0 <= x - SB[b] - j < D:
                    base[p, WIN * t + j] = 1.0
    return np.tile(base, (1, 4))


def _build_nc():
    nc = bacc.Bacc("TRN2", target_bir_lowering=False, debug=False,
                   num_devices=NCORES)
    fL_d = nc.dram_tensor("fLc", [C, HH, W], fp16, kind="ExternalInput").ap()
    fR_d = nc.dram_tensor("fRc", [C, HH, W], fp16, kind="ExternalInput").ap()
    mask_d = nc.dram_tensor("maskc", [128, 4 * 4 * WIN], fp16,
                            kind="ExternalInput").ap()
    # per load-batch: [p, (hpb, pr, hi, t, j')] -> 6KB DMA lines
    dump_d = nc.dram_tensor("dump", [NBATCH, 128, 16 * 4 * D], fp16,
                            kind="ExternalOutput").ap()

    with tile.TileContext(nc) as tc, ExitStack() as ctx:
        const_pool = ctx.enter_context(tc.tile_pool(name="const", bufs=1))
        in_pool = ctx.enter_context(tc.tile_pool(name="inp", bufs=3))
        raw_pool = ctx.enter_context(tc.tile_pool(name="raw", bufs=3))
        msk_pool = ctx.enter_context(tc.tile_pool(name="msk", bufs=3))
        f_pool = ctx.enter_context(tc.tile_pool(name="fold", bufs=3))
        mm_psum = ctx.enter_context(tc.tile_pool(name="mmps", bufs=2, space="PSUM"))

        mask_t = const_pool.tile([128, 4 * 4 * WIN], fp16)
        nc.scalar.dma_start(mask_t[:], mask_d)

        NHH = NH // 2
        for ib in range(NBATCH):
            h0 = ib * NH
            # h rows h0..h0+7 -> partitions 0:64, h0+8..h0+15 -> 64:128
            fl = in_pool.tile([128, NHH * W], fp16, tag="fl")
            fr = in_pool.tile([128, NHH * W], fp16, tag="fr")
            for half in range(2):
                nc.sync.dma_start(
                    fl[64 * half : 64 * half + 64, :]
                    .rearrange("c (h x) -> c h x", h=NHH),
                    fL_d[:, h0 + NHH * half : h0 + NHH * (half + 1), :],
                )
                nc.gpsimd.dma_start(
                    fr[64 * half : 64 * half + 64, :]
                    .rearrange("c (h x) -> c h x", h=NHH),
                    fR_d[:, h0 + NHH * half : h0 + NHH * (half + 1), :],
                )

            F = f_pool.tile([128, 16 * 4 * D], fp16)
            for hpb in range(4):
                # quad = pairs (2*hpb, 2*hpb+1) x hi in {0,1}; psum row
                # q = 2*pr + hi at cols [512q, 512q+448)
                ps = mm_psum.tile([128, 2048], fp32)
                for pr in range(2):
                    hp = 2 * hpb + pr
                    for t in range(4):
                        for Hc in range(2):
                            b = 2 * t + Hc
                            for hi in range(2):
                                q = 2 * pr + hi
                                nc.tensor.matmul(
                                    ps[64 * Hc : 64 * Hc + 64,
                                       512 * q + WIN * t : 512 * q + WIN * (t + 1)],
                                    fl[64 * hi : 64 * hi + 64,
                                       W * hp + GB * b : W * hp + GB * b + GB],
                                    fr[64 * hi : 64 * hi + 64,
                                       W * hp + SB[b] : W * hp + SB[b] + WIN],
                                    start=True,
                                    stop=True,
                                )
                # raw evacuation (ACT): psum fp32 -> SBUF fp16, no mask
                raw = raw_pool.tile([128, 4 * 4 * WIN], fp16)
                nc.scalar.copy(
                    raw[:].rearrange("p (q c) -> p q c", q=4),
                    ps[:].rearrange("p (q c) -> p q c", q=4)[:, :, 0 : 4 * WIN],
                )
                # band mask (DVE, 2x fp16)
                msk = msk_pool.tile([128, 4 * 4 * WIN], fp16)
                nc.vector.tensor_mul(msk[:], raw[:], mask_t[:])
                # fold 112 -> 48 (exactly one nonzero plane per output col)
                Tv = msk[:].rearrange("p (q t j) -> p q t j", q=4, t=4)
                Fv = (F[:, 4 * 4 * D * hpb : 4 * 4 * D * (hpb + 1)]
                      .rearrange("p (q t j) -> p q t j", q=4, t=4))
                with nc.allow_low_precision(reason="fold adds zeros"):
                    nc.vector.tensor_add(
                        Fv, Tv[:, :, :, 0:D], Tv[:, :, :, D : 2 * D]
                    )
                    nc.vector.tensor_add(
                        Fv[:, :, :, 0 : WIN - 2 * D],
                        Fv[:, :, :, 0 : WIN - 2 * D],
                        Tv[:, :, :, 2 * D : WIN],
                    )
            nc.scalar.dma_start(dump_d[ib], F[:])

    nc.compile()
    return nc


_NC_CACHE = None


def _get_nc():
    global _NC_CACHE
    if _NC_CACHE is None:
        _NC_CACHE = _build_nc()
    return _NC_CACHE


def make_in_maps(fL, fR):
    maskc = _make_mask()
    in_maps = []
    for core in range(NCORES):
        b, half = divmod(core, 2)
        sl = np.s_[b, :, half * HH : half * HH + HH, :]
        in_maps.append({
            "fLc": fL[sl].astype(np.float16),
            "fRc": fR[sl].astype(np.float16),
            "maskc": maskc,
        })
    return in_maps


_GATHER_CACHE = None


def _gather_tables():
    # out[d, h, x] = dump[ib, hp, P[x], 192*hi + CB[d, x]] * VALID[d, x]
    global _GATHER_CACHE
    if _GATHER_CACHE is None:
        xs = np.arange(W)
        ds = np.arange(D)
        bx = xs // GB
        ux = xs % GB
        Px = 64 * (bx % 2) + ux                          # [W]
        jabs = xs[None, :] - np.asarray(SB)[bx][None, :] - ds[:, None]  # [D, W]
        valid = (jabs >= 0) & (jabs < WIN)
        tb = bx // 2
        cb = D * tb[None, :] + np.where(valid, jabs, 0) % D             # [D, W]
        FI = Px[None, :] * (4 * D) + cb                  # [D, W] into [p, 192]
        _GATHER_CACHE = (FI.astype(np.int64), valid.astype(np.float32))
    return _GATHER_CACHE


def kernel(fL: np.ndarray, fR: np.ndarray) -> np.ndarray:
    fL = np.asarray(fL, dtype=np.float32)
    fR = np.asarray(fR, dtype=np.float32)
    nc = _get_nc()
    in_maps = make_in_maps(fL, fR)
    res = bass_utils.run_bass_kernel_spmd(nc, in_maps, core_ids=list(range(NCORES)))

    FI, valid = _gather_tables()
    out = np.empty((B, D, H, W), dtype=np.float32)
    for core in range(NCORES):
        b, half = divmod(core, 2)
        dump = res.results[core]["dump"]                 # [8, 128, 3072]
        # [ib, p, hpb, pr, hi, c] -> [h = (ib, hi, hpb, pr), p*192 + c]
        arr = (dump.reshape(NBATCH, 128, 4, 2, 2, 4 * D)
               .transpose(0, 4, 2, 3, 1, 5)
               .reshape(HH, 128 * 4 * D))
        g = arr[:, FI.reshape(-1)].reshape(HH, D, W).astype(np.float32)
        g *= valid[None, :, :]
        out[b, :, half * HH : half * HH + HH, :] = g.transpose(1, 0, 2)
    return out
